# revision 1
# baseline (speedup 1.0000x reference)
"""Chamfer distance kernel for Trainium2 (8 NeuronCores, SPMD).

Reference computation:
    p1 = pc1.reshape(-1, 3)  [N1=16384, 3]
    p2 = pc2.reshape(-1, 3)  [N2=16384, 3]
    d[i, j] = ||p1_i - p2_j||
    out = mean_j(min_i d[i,j]) + mean_i(min_j d[i,j])

Strategy:
  - Shard pc2 rows across 8 cores (2048 points each). Each core computes
    its full distance tile against all of pc1, in both orientations:
      A: [pc1-block=128 part, pc2-shard=2048 free] -> free-min = partial
         col-min (dist2 path), all-min'd across cores on the host.
      B: [pc2-block=128 part, pc1=16384 free]      -> free-min = complete
         row-min (dist1 path) for this core's shard.
  - sqrt is monotone, so mins are taken on squared distances; sqrt and the
    two means run on the host over 8*(16384+2048) partial mins (tiny).
  - SCALE*d2[i,j] = SCALE*(sq1[i] + sq2[j] - 2*dot(p1_i, p2_j)) is
    produced directly by one K=24 augmented matmul per tile: 18 rows give
    the double-compensated bf16 dot product (hi/mid/lo splits; error
    ~2.5e-7 instead of bf16's 2^-8 -- needed because the true NN d2 here
    is ~5e-5), 6 rows add sq1/sq2 (each split hi+mid+lo). PSUM then holds
    full fp32 SCALE*d2.
  - Each [128, 2048] PSUM tile becomes a [128, 1] row-min via one of two
    routes, interleaved 1:13 so PE (~474us), DVE (~469us) and ScalarE
    (~470us) all finish together at the PE's measured 1.2GHz floor:
    DVE tensor_reduce straight from PSUM (1x, ~2.24us), or ScalarE
    PSUM->SBUF fp16 copy (~1.85us; the x512 pre-scale keeps d2 mins in
    fp16 normal range) + DVE in-place pairwise-min folds at 2x + short
    reduce (~1.5us).
  - Matmul operands must sit at a 32-partition base (0/32/64), so the 8
    pc1 column-groups of 24 contraction rows are packed at bases
    {0,32,64} x 3 column-regions of [128, 6144] SBUF tensors; the small
    pc2-side operands are replicated at all 3 bases.
  - Walrus accepts only one sem-wait per compute instruction; Tile emits
    more on recycled tile slots. _legalize_waits strips transitively
    implied same-engine waits and splits the rest onto injected NoOps.
"""

import os
import sys

import numpy as np

for _p in ("/opt/trn_rl_repo",):
    if os.path.isdir(_p) and _p not in sys.path:
        sys.path.append(_p)

import ml_dtypes

import concourse.bass as bass
import concourse.mybir as mybir
import concourse.tile as tile
from concourse.bass_utils import run_bass_kernel_spmd

BF16 = ml_dtypes.bfloat16

N_CORES = 8
N1 = 16384            # total pc1 points
N_SHARD = 2048        # pc2 points per core
N_GROUPS = 8          # pc1 column-groups
GROUP_COLS = N1 // N_GROUPS  # 2048
K = 24                # augmented contraction depth
MM_N = 512            # matmul moving free dim (one PSUM bank of fp32)
SCALE = 512.0         # power-of-two scale on d2 (fp16 normal range)
DIRECT_EVERY = 13     # 1-in-N tiles reduced straight from PSUM (0 = none)
IN_COLS = 6 * GROUP_COLS + 2 * N_SHARD  # packed input columns (16384)

TRACE = False         # test harness can flip this for profiled runs
LAST_RESULTS = None   # stashed BassKernelResults for the test harness

_NC_CACHE = None


def _build_nc():
    """Build the per-core Bass module (same NEFF on all 8 cores)."""
    nc = bass.Bass(trn_type="TRN2")

    # Packed input, cols: [0:6144) p1w, [6144:12288) p1m,
    # [12288:14336) p2w, [14336:16384) p2m.
    inp = nc.dram_tensor("inp", [128, IN_COLS], mybir.dt.bfloat16,
                         kind="ExternalInput")
    # Packed output (SCALE*d2 partial mins):
    # mout[:, 0:128]   = m2[p, bi]: min over this core's pc2 shard for
    #                    pc1 point bi*128+p (host mins across cores).
    # mout[:, 128:256] = raw per-(bj, g) row-mins for pc2_shard point
    #                    bj*128+p vs pc1 group g (host mins over g).
    mout = nc.dram_tensor("mout", [128, N1 // 128 + N_SHARD // 128 * N_GROUPS],
                          mybir.dt.float32, kind="ExternalOutput")

    with tile.TileContext(nc) as tc:
        with (
            tc.tile_pool(name="ins", bufs=1) as ins_pool,
            tc.tile_pool(name="psum", bufs=2, space="PSUM") as psum_pool,
            tc.tile_pool(name="outs", bufs=1) as out_pool,
            tc.tile_pool(name="f16", bufs=8) as f16_pool,
        ):
            inp_sb = ins_pool.tile([128, IN_COLS], mybir.dt.bfloat16,
                                   tag="inp")
            # Four dma_starts land on different HWDGE queues and run
            # concurrently -> input load completes in about a quarter the
            # time (nothing else is running yet, so no port contention).
            q = IN_COLS // 4
            for qi in range(4):
                nc.sync.dma_start(inp_sb[:, qi * q:(qi + 1) * q],
                                  inp[:, qi * q:(qi + 1) * q])
            p1w_sb = inp_sb[:, 0:3 * GROUP_COLS]
            p1m_sb = inp_sb[:, 3 * GROUP_COLS:6 * GROUP_COLS]
            p2w_sb = inp_sb[:, 6 * GROUP_COLS:6 * GROUP_COLS + N_SHARD]
            p2m_sb = inp_sb[:, 6 * GROUP_COLS + N_SHARD:IN_COLS]

            mout_sb = out_pool.tile(
                [128, N1 // 128 + N_SHARD // 128 * N_GROUPS],
                mybir.dt.float32, tag="mout")
            m2_sb = mout_sb[:, 0:N1 // 128]
            # Raw per-(bj, g) row-mins; the min over g happens on the host
            # (saves 16 DVE second-level reduces + their per-bj dep chains).
            m1_sb = mout_sb[:, N1 // 128:]

            def grp(sb, g, c0, c1):
                """K-row slice of a group-packed pc1-side tensor."""
                q, h = g % 3, g // 3
                return sb[32 * q:32 * q + K, h * GROUP_COLS + c0:h * GROUP_COLS + c1]

            def rep(sb, g, c0, c1):
                """K-row slice of a base-replicated pc2-side tensor."""
                q = g % 3
                return sb[32 * q:32 * q + K, c0:c1]

            # Two ways to turn a PSUM tile into a [128,1] row-min:
            #  - DVE-direct: tensor_reduce(min) from PSUM fp32 (1x rate,
            #    ~2.24us/tile).
            #  - ACT-route: ScalarE copies PSUM -> SBUF fp16 (~1.85us; the
            #    x512 pre-scale keeps d2 mins in fp16 normal range), then
            #    DVE runs three in-place pairwise-min folds at 2x_1P rate
            #    plus a short 1x reduce (~1.44us total) -- 0.8us cheaper
            #    per tile than direct.
            # Route 1-in-DIRECT_EVERY direct so DVE and ACT drain at
            # matched rates.
            tile_idx = [0]

            def consume(pt, target):
                i = tile_idx[0]
                tile_idx[0] += 1
                if DIRECT_EVERY and i % DIRECT_EVERY < 1:
                    nc.vector.tensor_reduce(
                        out=target, in_=pt[:],
                        axis=mybir.AxisListType.X, op=mybir.AluOpType.min,
                    )
                else:
                    f16 = f16_pool.tile([128, N_SHARD], mybir.dt.float16,
                                        tag="f16")
                    nc.scalar.copy(f16[:], pt[:])
                    for half in (1024, 512, 256):
                        nc.vector.tensor_tensor(
                            out=f16[:, :half], in0=f16[:, :half],
                            in1=f16[:, half:2 * half],
                            op=mybir.AluOpType.min,
                        )
                    nc.vector.tensor_reduce(
                        out=target, in_=f16[:, :256],
                        axis=mybir.AxisListType.X, op=mybir.AluOpType.min,
                    )

            # Orientation A: 128 pc1-blocks; free dim = pc2 shard (2048).
            for bi in range(N1 // 128):
                g, b_in = divmod(bi, GROUP_COLS // 128)
                lhsT = grp(p1w_sb, g, b_in * 128, (b_in + 1) * 128)
                pt = psum_pool.tile([128, N_SHARD], mybir.dt.float32, tag="ps")
                for c in range(N_SHARD // MM_N):
                    nc.tensor.matmul(
                        pt[:, c * MM_N:(c + 1) * MM_N],
                        lhsT,
                        rep(p2m_sb, g, c * MM_N, (c + 1) * MM_N),
                        start=True, stop=True,
                    )
                consume(pt, m2_sb[:, bi:bi + 1])

            # m2 is complete after the A loop: ship it while B computes.
            nc.sync.dma_start(mout[:, 0:N1 // 128], m2_sb[:])

            # Orientation B: 16 pc2-blocks; free dim = all pc1 (8 groups x 2048).
            for bj in range(N_SHARD // 128):
                for g in range(N_GROUPS):
                    lhsT = rep(p2w_sb, g, bj * 128, (bj + 1) * 128)
                    pt = psum_pool.tile([128, GROUP_COLS], mybir.dt.float32,
                                        tag="ps")
                    for c in range(GROUP_COLS // MM_N):
                        nc.tensor.matmul(
                            pt[:, c * MM_N:(c + 1) * MM_N],
                            lhsT,
                            grp(p1m_sb, g, c * MM_N, (c + 1) * MM_N),
                            start=True, stop=True,
                        )
                    consume(pt, m1_sb[:, bj * N_GROUPS + g:bj * N_GROUPS + g + 1])

            nc.sync.dma_start(mout[:, N1 // 128:], m1_sb[:])

    _legalize_waits(nc)
    return nc


def _legalize_waits(nc):
    """Walrus's per-instruction structs carry at most one sem-wait, but
    Tile's sem assignment can emit several (slot-recycle WAR + input RAW).

    1. Same-engine waits are dropped when a cross-engine wait remains:
       engines execute in order and the cross-engine consumer they wait
       on transitively waited on those same-engine ticks.
    2. The kernel-tail Drain waits on every DMA queue + PE + DVE; all of
       it is transitively covered by the single output DMA.
    3. Any instruction still carrying N>1 waits gets N-1 same-engine
       NoOps injected right before it, one overflow wait each.
    """
    import concourse.mybir as mybir

    blocks = nc.m.functions[0].blocks

    # 1. same-engine strip
    for blk in blocks:
        for ins in blk.instructions:
            si = ins.sync_info
            if si is None or len(si.on_wait) <= 1 or not si.on_update:
                continue
            self_eng = si.on_update[0].ant_name.split("_")[0]
            keep = [w for w in si.on_wait
                    if w.ant_name.split("_")[0] != self_eng]
            if keep and len(keep) < len(si.on_wait):
                si.on_wait = keep
                ins.sync_info = si

    # 2. tail drain: keep only the output DMA queue's wait
    out_sems = set()
    for blk in blocks:
        for ins in blk.instructions:
            if type(ins).__name__ == "InstDMACopy" and ins.outs and \
                    getattr(ins.outs[0], "memref", "") == "mout":
                si = ins.sync_info
                for u in (si.on_update if si else []):
                    out_sems.add(u.ant_name)
    for blk in blocks:
        for ins in blk.instructions:
            if type(ins).__name__ != "InstDrain" or not out_sems:
                continue
            si = ins.sync_info
            if si is None or len(si.on_wait) <= 1:
                continue
            keep = [w for w in si.on_wait if w.ant_name in out_sems]
            if keep and len(keep) < len(si.on_wait):
                si.on_wait = keep
                ins.sync_info = si

    # 3. split remaining multi-waits onto same-engine NoOps
    eng_by_prefix = {
        "PE": mybir.EngineType.PE,
        "DVE": mybir.EngineType.DVE,
        "ACT": mybir.EngineType.Activation,
        "POOL": mybir.EngineType.Pool,
        "SP": mybir.EngineType.SP,
    }
    nop_id = [0]
    for blk in blocks:
        new_list = []
        changed = False
        for ins in blk.instructions:
            si = ins.sync_info
            if si is not None and len(si.on_wait) > 1:
                eng = getattr(ins, "engine", None)
                if eng is None and si.on_update:
                    eng = eng_by_prefix.get(
                        si.on_update[0].ant_name.split("_")[0])
                assert eng is not None, \
                    f"{ins.name}: cannot infer engine for wait split"
                waits = list(si.on_wait)
                for w in waits[:-1]:
                    nop_id[0] += 1
                    nop = mybir.InstNoOp(
                        name=f"I-waitnop-{nop_id[0]}", ins=[], outs=[],
                        engine=eng,
                        sync_info=mybir.SyncInfo(on_wait=[w], on_update=[]),
                    )
                    new_list.append(nop)
                si.on_wait = [waits[-1]]
                ins.sync_info = si
                changed = True
            new_list.append(ins)
        if changed:
            blk.instructions = new_list


def _split3(x):
    """fp32 -> three bf16 terms with x ~= h + m + l (residual ~2^-24 |x|)."""
    h = x.astype(BF16)
    r = x - h.astype(np.float32)
    m = r.astype(BF16)
    l = (r - m.astype(np.float32)).astype(BF16)
    return h, m, l


def _prep_side(p):
    """p: [N, 3] fp32 -> (weight_rows [24, N], moving_rows [24, N]).

    Row r of the weight side pairs with row r of the other cloud's moving
    side; the contraction sums, per coordinate, the six hi/mid/lo product
    terms of magnitude >= ~2^-17 (double-compensated bf16 dot, error
    ~2.5e-7), plus three hi/mid/lo rows for each side's |p|^2. The weight
    side carries SCALE (a power of two), so PSUM holds SCALE*d2 exactly
    scaled -- keeping d2 row-mins (~2.5e-5 here) inside fp16 normal range
    for the ACT-routed fp16 evacuation path.
    """
    x, y, z = p[:, 0], p[:, 1], p[:, 2]
    sq = (x * x + y * y + z * z).astype(np.float32)
    w_rows, m_rows = [], []
    for c in (x, y, z):
        h, m, l = _split3(c)
        # (W, M) pairs: (h,h) (m,h) (h,m) (l,h) (m,m) (h,l)
        w_rows += [-2 * SCALE * h, -2 * SCALE * m, -2 * SCALE * h,
                   -2 * SCALE * l, -2 * SCALE * m, -2 * SCALE * h]
        m_rows += [h, h, m, h, m, l]
    ones = np.ones_like(sq)
    w_rows += [SCALE * ones] * 3 + list(_split3(SCALE * sq))
    m_rows += list(_split3(sq)) + [ones] * 3
    return (np.stack(w_rows).astype(BF16), np.stack(m_rows).astype(BF16))


def _group_pack(rows13):
    """[13, N1] -> [128, 6144]: group g at partition base 32*(g%3),
    column region g//3 (AP base partition must be in {0,32,64})."""
    out = np.zeros((128, 3 * GROUP_COLS), dtype=BF16)
    for g in range(N_GROUPS):
        q, h = g % 3, g // 3
        out[32 * q:32 * q + K, h * GROUP_COLS:(h + 1) * GROUP_COLS] = \
            rows13[:, g * GROUP_COLS:(g + 1) * GROUP_COLS]
    return out


def _rep_pack(rows13):
    """[13, N_SHARD] -> [128, N_SHARD]: replicated at bases 0/32/64."""
    out = np.zeros((128, N_SHARD), dtype=BF16)
    for q in range(3):
        out[32 * q:32 * q + K, :] = rows13
    return out


def kernel(pc1, pc2):
    global _NC_CACHE, LAST_RESULTS
    p1 = np.asarray(pc1, dtype=np.float32).reshape(-1, 3)
    p2 = np.asarray(pc2, dtype=np.float32).reshape(-1, 3)
    assert p1.shape == (N1, 3) and p2.shape == (N_CORES * N_SHARD, 3)

    w1, m1rows = _prep_side(p1)
    p1w_np = _group_pack(w1)
    p1m_np = _group_pack(m1rows)

    in_maps = []
    for c in range(N_CORES):
        shard = p2[c * N_SHARD:(c + 1) * N_SHARD]
        w2, m2rows = _prep_side(shard)
        packed = np.concatenate(
            [p1w_np, p1m_np, _rep_pack(w2), _rep_pack(m2rows)], axis=1)
        in_maps.append({"inp": np.ascontiguousarray(packed)})

    if _NC_CACHE is None:
        _NC_CACHE = _build_nc()

    res = run_bass_kernel_spmd(
        _NC_CACHE, in_maps, core_ids=list(range(N_CORES)), trace=TRACE,
    )
    LAST_RESULTS = res

    # m1 per core: complete row-mins of d2 for its 2048 pc2 points.
    # m2 per core: partial col-mins of d2 over its shard -> min across cores.
    nb2 = N1 // 128
    d2_1 = np.concatenate(
        [r["mout"][:, nb2:].reshape(128, N_SHARD // 128, N_GROUPS)
         .min(axis=2).T.reshape(-1) for r in res.results])        # [16384] pc2-major
    d2_2 = np.min(
        np.stack([r["mout"][:, :nb2].T.reshape(-1) for r in res.results]),
        axis=0)                                                   # [16384]

    dist1 = np.sqrt(np.maximum(d2_1 / SCALE, 0.0))
    dist2 = np.sqrt(np.maximum(d2_2 / SCALE, 0.0))
    return np.asarray(dist1.mean() + dist2.mean(), dtype=np.float32)



# revision 6
# speedup vs baseline: 15.4842x; 15.4842x over previous
"""Chamfer distance kernel for Trainium2 (8 NeuronCores, SPMD).

Reference computation:
    p1 = pc1.reshape(-1, 3)  [N1=16384, 3]
    p2 = pc2.reshape(-1, 3)  [N2=16384, 3]
    d[i, j] = ||p1_i - p2_j||
    out = mean_j(min_i d[i,j]) + mean_i(min_j d[i,j])

Strategy (sorted-window candidate search + exact host certification):
  - Both clouds are sorted by x on the host. For 16384 standard-normal
    points the NN distance is ~0.007 while a +-256-rank window in sorted
    x order spans ~0.1+ in x, so the true NN of a point lies inside a
    WIN=512 window around its own sorted rank for all but a handful of
    points. Each 128-point block of one cloud gets one K=24 matmul
    against its window of the other cloud: [24,128].T @ [24,WIN] ->
    PSUM [128, WIN] holding exact-ish d2 (double-compensated bf16, abs
    err ~5e-7), then a single DVE min-reduction -> [128,1] row-min.
  - Both directions are computed this way (32 tiles/core total); pc2
    blocks 16c..16c+15 and pc1 blocks 16c..16c+15 live on core c, so
    every block's min is complete on its core: no accumulator, no
    partition-axis reduction, no cross-core combine.
  - The moving-side operands are padded with far-away points (x=1024)
    so every block uses the same window offsets (uniform SPMD kernel);
    pad distances are ~3e6, never the min.
  - Host certification makes the result exact: a windowed min d is
    provably the true min when d <= x-distance to the nearest excluded
    sorted neighbor. The ~100-300 points failing that certificate get
    their min recomputed exactly in numpy (negligible host work).
  - Four blocks share one 4-bank PSUM tile; a single DVE tensor_reduce
    over a [128, (4, 512)] view yields all four block-mins at the fp32
    PSUM streaming rate with only 8 DVE instructions per core.
  - Walrus accepts only one sem-wait per compute instruction; Tile
    emits more on recycled slots. _legalize_waits strips transitively
    implied same-engine waits and splits the rest onto injected NoOps.
"""

import os
import sys

import numpy as np

for _p in ("/opt/trn_rl_repo",):
    if os.path.isdir(_p) and _p not in sys.path:
        sys.path.append(_p)

import ml_dtypes

import concourse.bass as bass
import concourse.mybir as mybir
import concourse.tile as tile
from concourse.bass_utils import run_bass_kernel_spmd

BF16 = ml_dtypes.bfloat16

N_CORES = 8
N = 16384             # points per cloud
K = 24                # augmented contraction depth (compensated bf16)
WIN = 512             # candidate window (sorted ranks) per 128-block
HALF = WIN // 2
SPAN = 1920 + WIN     # per-core moving-side span: 15*128 + WIN
NBLK = 16             # 128-point blocks per core per direction
PAD = 1024.0          # far-point coordinate for window padding
GRP = 4               # blocks per PSUM tile / DVE reduce

TRACE = False         # test harness can flip this for profiled runs
LAST_RESULTS = None   # stashed BassKernelResults for the test harness

_NC_CACHE = None


def _build_nc():
    """Build the per-core Bass module (same NEFF on all 8 cores)."""
    nc = bass.Bass(trn_type="TRN2")

    # Packed input columns: [w2 2048 | m1 SPAN | w1 2048 | m2 SPAN]
    # w2/w1: weight-side rows of this core's own pc2/pc1 blocks.
    # m1/m2: moving-side rows of the padded pc1/pc2 candidate spans.
    cols = 2 * (2048 + SPAN)
    inp = nc.dram_tensor("inp", [K, cols], mybir.dt.bfloat16,
                         kind="ExternalInput")
    # mout[:, bj]      = min_d2 for pc2 point 128*(16c+bj)+p  (dir 1)
    # mout[:, 16+bj]   = min_d2 for pc1 point 128*(16c+bj)+p  (dir 2)
    mout = nc.dram_tensor("mout", [128, 2 * NBLK], mybir.dt.float32,
                          kind="ExternalOutput")

    with tile.TileContext(nc) as tc:
        with (
            tc.tile_pool(name="ins", bufs=1) as ins_pool,
            tc.tile_pool(name="psum", bufs=2, space="PSUM") as psum_pool,
            tc.tile_pool(name="outs", bufs=1) as out_pool,
        ):
            w2_sb = ins_pool.tile([K, 2048], mybir.dt.bfloat16, tag="w2")
            m1_sb = ins_pool.tile([K, SPAN], mybir.dt.bfloat16, tag="m1")
            w1_sb = ins_pool.tile([K, 2048], mybir.dt.bfloat16, tag="w1")
            m2_sb = ins_pool.tile([K, SPAN], mybir.dt.bfloat16, tag="m2")
            c0 = 0
            for seg, width in ((w2_sb, 2048), (m1_sb, SPAN),
                               (w1_sb, 2048), (m2_sb, SPAN)):
                h = width // 2
                nc.sync.dma_start(seg[:, 0:h], inp[:, c0:c0 + h])
                nc.sync.dma_start(seg[:, h:width], inp[:, c0 + h:c0 + width])
                c0 += width

            mo = out_pool.tile([128, 2 * NBLK], mybir.dt.float32, tag="mo")

            # 32 blocks (16 per direction), GRP per PSUM tile; one DVE
            # min-reduce per group covers all its blocks.
            for g in range(2 * NBLK // GRP):
                pt = psum_pool.tile([128, GRP * WIN], mybir.dt.float32,
                                    tag="ps")
                for k in range(GRP):
                    i = g * GRP + k
                    wsb, msb, bj = (
                        (w2_sb, m1_sb, i) if i < NBLK
                        else (w1_sb, m2_sb, i - NBLK))
                    nc.tensor.matmul(
                        pt[:, k * WIN:(k + 1) * WIN],
                        wsb[:, 128 * bj:128 * bj + 128],
                        msb[:, 128 * bj:128 * bj + WIN],
                        start=True, stop=True,
                    )
                nc.vector.tensor_reduce(
                    out=mo[:, g * GRP:(g + 1) * GRP],
                    in_=pt[:].rearrange("p (a b) -> p a b", a=GRP),
                    axis=mybir.AxisListType.X, op=mybir.AluOpType.min,
                )

            nc.sync.dma_start(mout[:], mo[:])

    _legalize_waits(nc)
    return nc


def _legalize_waits(nc):
    """Walrus's per-instruction structs carry at most one sem-wait, but
    Tile's sem assignment can emit several (slot-recycle WAR + input RAW).

    1. Same-engine waits are dropped when a cross-engine wait remains:
       engines execute in order and the cross-engine consumer they wait
       on transitively waited on those same-engine ticks.
    2. The kernel-tail Drain waits on every DMA queue + PE + DVE; all of
       it is transitively covered by the single output DMA.
    3. Any instruction still carrying N>1 waits gets N-1 same-engine
       NoOps injected right before it, one overflow wait each.
    """
    import concourse.mybir as mybir

    blocks = nc.m.functions[0].blocks

    # 1. same-engine strip
    for blk in blocks:
        for ins in blk.instructions:
            si = ins.sync_info
            if si is None or len(si.on_wait) <= 1 or not si.on_update:
                continue
            self_eng = si.on_update[0].ant_name.split("_")[0]
            keep = [w for w in si.on_wait
                    if w.ant_name.split("_")[0] != self_eng]
            if keep and len(keep) < len(si.on_wait):
                si.on_wait = keep
                ins.sync_info = si

    # 2. tail drain: keep only the output DMA queue's wait
    out_sems = set()
    for blk in blocks:
        for ins in blk.instructions:
            if type(ins).__name__ == "InstDMACopy" and ins.outs and \
                    getattr(ins.outs[0], "memref", "") == "mout":
                si = ins.sync_info
                for u in (si.on_update if si else []):
                    out_sems.add(u.ant_name)
    for blk in blocks:
        for ins in blk.instructions:
            if type(ins).__name__ != "InstDrain" or not out_sems:
                continue
            si = ins.sync_info
            if si is None or len(si.on_wait) <= 1:
                continue
            keep = [w for w in si.on_wait if w.ant_name in out_sems]
            if keep and len(keep) < len(si.on_wait):
                si.on_wait = keep
                ins.sync_info = si

    # 3. split remaining multi-waits onto same-engine NoOps
    eng_by_prefix = {
        "PE": mybir.EngineType.PE,
        "DVE": mybir.EngineType.DVE,
        "ACT": mybir.EngineType.Activation,
        "POOL": mybir.EngineType.Pool,
        "SP": mybir.EngineType.SP,
    }
    nop_id = [0]
    for blk in blocks:
        new_list = []
        changed = False
        for ins in blk.instructions:
            si = ins.sync_info
            if si is not None and len(si.on_wait) > 1:
                eng = getattr(ins, "engine", None)
                if eng is None and si.on_update:
                    eng = eng_by_prefix.get(
                        si.on_update[0].ant_name.split("_")[0])
                assert eng is not None, \
                    f"{ins.name}: cannot infer engine for wait split"
                waits = list(si.on_wait)
                for w in waits[:-1]:
                    nop_id[0] += 1
                    nop = mybir.InstNoOp(
                        name=f"I-waitnop-{nop_id[0]}", ins=[], outs=[],
                        engine=eng,
                        sync_info=mybir.SyncInfo(on_wait=[w], on_update=[]),
                    )
                    new_list.append(nop)
                si.on_wait = [waits[-1]]
                ins.sync_info = si
                changed = True
            new_list.append(ins)
        if changed:
            blk.instructions = new_list


def _split3(x):
    """fp32 -> three bf16 terms with x ~= h + m + l (residual ~2^-24 |x|)."""
    h = x.astype(BF16)
    r = x - h.astype(np.float32)
    m = r.astype(BF16)
    l = (r - m.astype(np.float32)).astype(BF16)
    return h, m, l


def _prep_side(p):
    """p: [N, 3] fp32 -> (weight_rows [24, N], moving_rows [24, N]).

    Row r of the weight side pairs with row r of the other cloud's moving
    side; the contraction sums, per coordinate, the six hi/mid/lo product
    terms of magnitude >= ~2^-17 (double-compensated bf16 dot, error
    ~2.5e-7), plus three hi/mid/lo rows for each side's |p|^2, so PSUM
    holds d2 = |w|^2 + |m|^2 - 2 w.m in nearly-fp32 precision.
    """
    x, y, z = p[:, 0], p[:, 1], p[:, 2]
    sq = (x * x + y * y + z * z).astype(np.float32)
    w_rows, m_rows = [], []
    for c in (x, y, z):
        h, m, l = _split3(c)
        # (W, M) pairs: (h,h) (m,h) (h,m) (l,h) (m,m) (h,l)
        w_rows += [-2 * h, -2 * m, -2 * h, -2 * l, -2 * m, -2 * h]
        m_rows += [h, h, m, h, m, l]
    ones = np.ones_like(sq)
    w_rows += [ones] * 3 + list(_split3(sq))
    m_rows += list(_split3(sq)) + [ones] * 3
    return (np.stack(w_rows).astype(BF16), np.stack(m_rows).astype(BF16))


def _exact_min_d2(q, ref):
    """Exact per-point min ||q_i - ref_j||^2 over all ref (host patch)."""
    d2 = ((q * q).sum(1)[:, None] + (ref * ref).sum(1)[None, :]
          - 2.0 * (q @ ref.T))
    return np.maximum(d2, 0.0).min(1)


def _certify_patch(d2_min, qs, refs):
    """Windowed mins -> exact mins.

    d2_min[j] is the min over sorted-ref ranks [128b+64-HALF, 128b+64+HALF)
    (clipped), b = j // 128. The min is provably exact when
    sqrt(d2) <= x-distance to the nearest excluded sorted ref point;
    everything else is recomputed exactly.
    """
    n = len(qs)
    j = np.arange(n)
    b = j // 128
    lo = 128 * b + 64 - HALF
    hi = 128 * b + 64 + HALF
    xq, xr = qs[:, 0], refs[:, 0]
    guard_lo = np.where(lo > 0, xq - xr[np.clip(lo - 1, 0, n - 1)], np.inf)
    guard_hi = np.where(hi < n, xr[np.clip(hi, 0, n - 1)] - xq, np.inf)
    guard = np.minimum(guard_lo, guard_hi)
    d = np.sqrt(np.maximum(d2_min, 0.0))
    fail = d > guard - 1e-6
    if fail.any():
        idx = np.where(fail)[0]
        d2_min = d2_min.copy()
        d2_min[idx] = _exact_min_d2(qs[idx], refs)
    return d2_min


def kernel(pc1, pc2):
    global _NC_CACHE, LAST_RESULTS
    p1 = np.ascontiguousarray(np.asarray(pc1, dtype=np.float32).reshape(-1, 3))
    p2 = np.ascontiguousarray(np.asarray(pc2, dtype=np.float32).reshape(-1, 3))
    assert p1.shape == (N, 3) and p2.shape == (N, 3)

    s1 = np.argsort(p1[:, 0], kind="stable")
    s2 = np.argsort(p2[:, 0], kind="stable")
    p1s, p2s = p1[s1], p2[s2]

    pad = np.full((HALF, 3), PAD, dtype=np.float32)
    p1pad = np.concatenate([pad, p1s, pad])
    p2pad = np.concatenate([pad, p2s, pad])

    w1, _ = _prep_side(p1s)
    w2, _ = _prep_side(p2s)
    _, m1 = _prep_side(p1pad)
    _, m2 = _prep_side(p2pad)

    in_maps = []
    for c in range(N_CORES):
        a = 2048 * c
        packed = np.concatenate(
            [w2[:, a:a + 2048], m1[:, a + 64:a + 64 + SPAN],
             w1[:, a:a + 2048], m2[:, a + 64:a + 64 + SPAN]], axis=1)
        in_maps.append({"inp": np.ascontiguousarray(packed)})

    if _NC_CACHE is None:
        _NC_CACHE = _build_nc()

    res = run_bass_kernel_spmd(
        _NC_CACHE, in_maps, core_ids=list(range(N_CORES)), trace=TRACE,
    )
    LAST_RESULTS = res

    d2_1 = np.concatenate([r["mout"][:, 0:NBLK].T.reshape(-1)
                           for r in res.results])   # sorted-pc2 order
    d2_2 = np.concatenate([r["mout"][:, NBLK:2 * NBLK].T.reshape(-1)
                           for r in res.results])   # sorted-pc1 order

    d2_1 = _certify_patch(d2_1, p2s, p1s)
    d2_2 = _certify_patch(d2_2, p1s, p2s)

    dist1 = np.sqrt(np.maximum(d2_1, 0.0))
    dist2 = np.sqrt(np.maximum(d2_2, 0.0))
    return np.asarray(dist1.mean() + dist2.mean(), dtype=np.float32)
